# revision 13
# baseline (speedup 1.0000x reference)
"""ContrastiveLoss Trainium2 kernel — adjacency-paired gathers, v2.

Data-parallel over 8 cores = 4 batches x 2 A-row-range halves: core (b, h)
handles batch b samples whose outA row index falls in [h*N/2, (h+1)*N/2).
Row-range sharding doubles the A-side index density (27500 draws over
153600 rows), which raises the adjacent-pair yield of the greedy pairing.

Gather primitive: gpsimd indirect1d — one index per SBUF partition per
instruction, streaming the out free dim contiguously from that row
(hardware-verified; ~1.41 us per instruction regardless of stream length).
Instruction count is the whole game: a group of 128 same-gap pairs costs
3 instructions (1 paired-side stream + 2 partner columns) vs 4 as singles.

v2 pads pair groups to the max count over cores with weight-0 dummy pairs
(rows (0, gap)) instead of truncating to the min, so every core's full
pair yield is used while all 8 cores share one NEFF shape.

Slot layout per core (COLS columns x 128 partitions): one region per
(side, gap) round, block width gap+1, then [singles + pads].
Interior cells of gap>=2 blocks are zeroed by strided memsets (weights
there are 0; the memset keeps stale SBUF NaN/Inf out of the weighted sum).
Per-slot f32 weights wm/wn (1.0 for match/nonmatch, 0 for pads/dummies):
dist -> match partial = sum dist*wm, nonmatch partial = sum relu(0.5-dist)*wn,
partition-reduced by a ones-vector matmul.
"""

import os

import numpy as np

B, N, D = 4, 307200, 16
M_MATCH, M_NONMATCH = 5000, 50000
MARGIN = 0.5
NON_MATCH_WEIGHT = 1.0
NCORES = 8

P = 128
CHUNK = 32  # compute-chunk width in columns (straddling pair blocks are
# safe: the tile framework tracks dependencies per byte range)

LAST_EXEC_NS = None
_CACHE = {}


def _run_scan(vals, order, gaps):
    """Greedy run-forming over `order` (sorted by vals): take k consecutive
    sorted entries whose value deltas match `gaps` exactly."""
    k = len(gaps) + 1
    runs = []
    i = 0
    n = len(order)
    while i <= n - k:
        ok = True
        for j in range(k - 1):
            if vals[order[i + j + 1]] != vals[order[i + j]] + gaps[j]:
                ok = False
                break
        if ok:
            runs.append(tuple(order[i + j] for j in range(k)))
            i += k
        else:
            i += 1
    return runs


# rounds: (side, gaps) — one descriptor streams the run's rows on `side`,
# the other side gathers singles at the run's real cells. Triples first
# (4 instr / 384 samples), then pairs (3 / 256); leftovers are singles
# (2 / 128). Side 0 runs on A-rows (density 2x from a-range sharding).
ROUNDS = (
    [(0, (g1, g2)) for g1 in range(1, 7) for g2 in range(1, 7)]
    + [(1, (g1, g2)) for g1 in range(1, 3) for g2 in range(1, 3)]
    + [(side, (gap,)) for gap in range(1, 17) for side in (0, 1)]
)


def _locs(gaps):
    out = [0]
    for g in gaps:
        out.append(out[-1] + g)
    return out


def _plan_all(cores):
    """Global cascade planner: per round, each core greedily pairs its
    still-available samples; the shared group count is the median core's
    yield (cores above keep extras available for later rounds, cores
    below pad with weight-0 dummy pairs). Returns per-core taken pair
    lists per round, the shared group counts, and per-core used masks."""
    used = [np.zeros(len(a), np.bool_) for a, _ in cores]
    taken = [[] for _ in cores]
    ns = []
    for side, gaps in ROUNDS:
        pls = []
        for ci, (a, b) in enumerate(cores):
            vals = a if side == 0 else b
            rem = np.where(~used[ci])[0]
            order = rem[np.argsort(vals[rem], kind="stable")]
            pls.append(_run_scan(vals, order, gaps))
        cnts = sorted(len(p) for p in pls)
        n = cnts[5] // P  # p75 over cores: best measured instr tradeoff
        ns.append(n)
        for ci, pl in enumerate(pls):
            take = pl[: n * P]
            for tup in take:
                for s in tup:
                    used[ci][s] = True
            taken[ci].append(take)
    return taken, ns, used


def _build_nc(ns, COLS):
    import concourse.bacc as bacc
    import concourse.mybir as mybir
    import concourse.tile as tile
    from concourse import bass

    f32 = mybir.dt.float32
    i32 = mybir.dt.int32
    X = mybir.AxisListType.X
    ADD = mybir.AluOpType.add
    MULT = mybir.AluOpType.mult
    Relu = mybir.ActivationFunctionType.Relu

    nc = bacc.Bacc(
        "TRN2", target_bir_lowering=False, debug=False, num_swdge_queues=2
    )
    eA = nc.dram_tensor("eA", (N, D), f32, kind="ExternalInput")
    eB = nc.dram_tensor("eB", (N, D), f32, kind="ExternalInput")
    ia = nc.dram_tensor("ia", (P, COLS), i32, kind="ExternalInput")
    ib = nc.dram_tensor("ib", (P, COLS), i32, kind="ExternalInput")
    wm = nc.dram_tensor("wm", (P, COLS), f32, kind="ExternalInput")
    wn = nc.dram_tensor("wn", (P, COLS), f32, kind="ExternalInput")
    out = nc.dram_tensor("out", (1, 2), f32, kind="ExternalOutput")

    qctr = [0]

    def gather(dst_ap, src, idx_ap):
        inst = nc.gpsimd.indirect_dma_start(
            out=dst_ap,
            out_offset=None,
            in_=src.ap(),
            in_offset=bass.IndirectOffsetOnAxis(ap=idx_ap, axis=0),
        )
        if qctr[0] % 2:
            inst.ins.queue = "qPoolDynamic1"
        qctr[0] += 1

    with tile.TileContext(nc) as tc:
        with (
            tc.tile_pool(name="io", bufs=1) as iop,
            tc.tile_pool(name="gath", bufs=1) as gp,
            tc.tile_pool(name="cmp", bufs=4) as cp,
            tc.tile_pool(name="psum", bufs=1, space="PSUM") as pp,
        ):
            # first compute chunk's indices load first
            c0 = min(CHUNK, COLS)
            ia_t = iop.tile([P, COLS], i32)
            nc.sync.dma_start(ia_t[:, :c0], ia.ap()[:, :c0])
            ib_t = iop.tile([P, COLS], i32)
            nc.sync.dma_start(ib_t[:, :c0], ib.ap()[:, :c0])
            if COLS > c0:
                nc.sync.dma_start(ia_t[:, c0:], ia.ap()[:, c0:])
                nc.sync.dma_start(ib_t[:, c0:], ib.ap()[:, c0:])
            wm_t = iop.tile([P, COLS], f32)
            nc.sync.dma_start(wm_t[:], wm.ap())
            wn_t = iop.tile([P, COLS], f32)
            nc.sync.dma_start(wn_t[:], wn.ap())
            margin_t = iop.tile([P, 1], f32)
            nc.vector.memset(margin_t[:], MARGIN)

            gA = gp.tile([P, COLS * D], f32)
            gB = gp.tile([P, COLS * D], f32)
            dist = gp.tile([P, COLS], f32)
            hng = gp.tile([P, COLS], f32)
            sums = gp.tile([P, 2], f32)
            nc.vector.memset(sums[:], 0.0)

            regions = []  # (start, end, side, w, locs) in ROUNDS order
            base = 0
            for (side, gaps), n in zip(ROUNDS, ns):
                w = sum(gaps) + 1
                regions.append((base, base + w * n, side, w, _locs(gaps)))
                base += w * n

            # run blocks leave interior cells (between real cells) unwritten
            # on the single-descriptor side; zero them so stale SBUF can't
            # poison the (weight-0) distance with NaN/Inf.
            for start, end, side, w, locs in regions:
                if end <= start:
                    continue
                buf = gB if side == 0 else gA
                view = buf[:, start * D : end * D].rearrange(
                    "p (m c) -> p m c", c=w * D
                )
                for l0, l1 in zip(locs, locs[1:]):
                    if l1 > l0 + 1:
                        nc.vector.memset(
                            view[:, :, (l0 + 1) * D : l1 * D], 0.0
                        )

            def emit_col(c):
                for start, end, side, w, locs in regions:
                    if c < end:
                        loc = (c - start) % w
                        pair_src = (gA, eA, ia_t) if side == 0 else (gB, eB, ib_t)
                        sgl_src = (gB, eB, ib_t) if side == 0 else (gA, eA, ia_t)
                        if loc == 0:
                            g_t, e_t, i_t = pair_src
                            gather(
                                g_t[:, c * D : (c + w) * D],
                                e_t,
                                i_t[:, c : c + 1],
                            )
                        if loc in locs:
                            g_t, e_t, i_t = sgl_src
                            gather(
                                g_t[:, c * D : (c + 1) * D], e_t, i_t[:, c : c + 1]
                            )
                        return
                gather(gA[:, c * D : (c + 1) * D], eA, ia_t[:, c : c + 1])
                gather(gB[:, c * D : (c + 1) * D], eB, ib_t[:, c : c + 1])

            for cs in range(0, COLS, CHUNK):
                ce = min(cs + CHUNK, COLS)
                for c in range(cs, ce):
                    emit_col(c)
                w = ce - cs
                nd = cp.tile([P, CHUNK * D], f32, tag="nd")
                nc.vector.tensor_sub(
                    nd[:, : w * D], gA[:, cs * D : ce * D], gB[:, cs * D : ce * D]
                )
                nsq = cp.tile([P, CHUNK * D], f32, tag="nsq")
                nc.scalar.square(nsq[:, : w * D], nd[:, : w * D])
                nc.vector.tensor_reduce(
                    dist[:, cs:ce],
                    nsq[:, : w * D].rearrange("p (s d) -> p s d", d=D),
                    axis=X,
                    op=ADD,
                )
                nc.scalar.activation(
                    hng[:, cs:ce],
                    dist[:, cs:ce],
                    Relu,
                    bias=margin_t[:],
                    scale=-1.0,
                )
                # weighted partials accumulated per chunk (keeps the
                # post-gather tail short)
                md = cp.tile([P, CHUNK], f32, tag="md")
                nc.vector.tensor_tensor(
                    out=md[:, :w], in0=dist[:, cs:ce], in1=wm_t[:, cs:ce],
                    op=MULT,
                )
                rr = cp.tile([P, 2], f32, tag="rr")
                nc.vector.tensor_reduce(rr[:, 0:1], md[:, :w], axis=X, op=ADD)
                nh = cp.tile([P, CHUNK], f32, tag="nh")
                nc.vector.tensor_tensor(
                    out=nh[:, :w], in0=hng[:, cs:ce], in1=wn_t[:, cs:ce],
                    op=MULT,
                )
                nc.vector.tensor_reduce(rr[:, 1:2], nh[:, :w], axis=X, op=ADD)
                nc.vector.tensor_tensor(
                    out=sums[:], in0=sums[:], in1=rr[:], op=ADD
                )

            ones = gp.tile([P, 1], f32)
            nc.vector.memset(ones[:], 1.0)
            acc = pp.tile([1, 2], f32, space="PSUM")
            nc.tensor.matmul(acc[:], lhsT=ones[:], rhs=sums[:], start=True, stop=True)
            res = gp.tile([1, 2], f32)
            nc.vector.tensor_copy(res[:], acc[:])
            nc.sync.dma_start(out.ap(), res[:])

    nc.compile()
    return nc


def _in_maps(outA, outB, matchA, matchB, nonMatchA, nonMatchB):
    outA = np.ascontiguousarray(np.asarray(outA, dtype=np.float32))
    outB = np.ascontiguousarray(np.asarray(outB, dtype=np.float32))
    matchA = np.asarray(matchA).astype(np.int64)
    matchB = np.asarray(matchB).astype(np.int64)
    nonMatchA = np.asarray(nonMatchA).astype(np.int64)
    nonMatchB = np.asarray(nonMatchB).astype(np.int64)

    cores = []
    meta = []
    for c in range(NCORES):
        b, h = c // 2, c % 2
        a_all = np.concatenate([matchA[b], nonMatchA[b]])
        b_all = np.concatenate([matchB[b], nonMatchB[b]])
        ismatch_all = np.zeros(len(a_all), np.bool_)
        ismatch_all[: matchA.shape[1]] = True
        sel = (a_all >= h * (N // 2)) & (a_all < (h + 1) * (N // 2))
        cores.append((a_all[sel], b_all[sel]))
        meta.append(ismatch_all[sel])

    taken, ns, used_masks = _plan_all(cores)
    nScols = max(
        -(-int((~used_masks[ci]).sum()) // P) for ci in range(NCORES)
    )
    COLS = sum(n * (sum(gaps) + 1) for n, (_, gaps) in zip(ns, ROUNDS)) + nScols

    maps = []
    for ci in range(NCORES):
        a, bb = cores[ci]
        ismatch = meta[ci]
        plists = taken[ci]
        used = used_masks[ci]
        b = ci // 2
        ia = np.zeros((P, COLS), np.int32)
        ib = np.zeros((P, COLS), np.int32)
        wm = np.zeros((P, COLS), np.float32)
        wn = np.zeros((P, COLS), np.float32)

        def place(s, p, col):
            ia[p, col] = a[s]
            ib[p, col] = bb[s]
            wm[p, col] = 1.0 if ismatch[s] else 0.0
            wn[p, col] = 0.0 if ismatch[s] else 1.0

        base = 0
        for r, ((side, gaps), n) in enumerate(zip(ROUNDS, ns)):
            w = sum(gaps) + 1
            locs = _locs(gaps)
            pl = plists[r]
            iv = ia if side == 0 else ib
            for t in range(n * P):
                k, p = divmod(t, P)
                c0 = base + w * k
                if t < len(pl):
                    for s, loc in zip(pl[t], locs):
                        place(s, p, c0 + loc)
                else:
                    # dummy run: rows equal to the loc offsets, weight 0
                    for loc in locs:
                        iv[p, c0 + loc] = loc
            for k in range(n):
                cc = base + w * k
                for loc in locs[1:]:
                    assert np.all(iv[:, cc + loc] == iv[:, cc] + loc)
            base += w * n
        singles = np.where(~used)[0]
        for i, s in enumerate(singles):
            place(s, i % P, base + i // P)

        maps.append(
            {
                "eA": outA[b],
                "eB": outB[b],
                "ia": ia,
                "ib": ib,
                "wm": wm,
                "wn": wn,
            }
        )
    return maps, ns, COLS


def kernel(outA, outB, matchA, matchB, nonMatchA, nonMatchB):
    global LAST_EXEC_NS
    from concourse import bass_utils

    maps, ns, COLS = _in_maps(
        outA, outB, matchA, matchB, nonMatchA, nonMatchB
    )
    ck = (tuple(ns), COLS)
    if _CACHE.get("key") != ck:
        _CACHE["nc"] = _build_nc(ns, COLS)
        _CACHE["key"] = ck
    nc = _CACHE["nc"]

    kwargs = {}
    if os.environ.get("KERNEL_TRACE", "0") == "1":
        kwargs["trace"] = True
    r = bass_utils.run_bass_kernel_spmd(
        nc, maps, core_ids=list(range(NCORES)), **kwargs
    )
    LAST_EXEC_NS = r.exec_time_ns

    partial = np.stack(
        [np.asarray(r.results[c]["out"]).ravel() for c in range(NCORES)]
    )
    match_loss = partial[:, 0].sum(dtype=np.float64) / M_MATCH
    nonmatch_loss = (
        NON_MATCH_WEIGHT * partial[:, 1].sum(dtype=np.float64) / M_NONMATCH
    )
    contrastive = match_loss + nonmatch_loss
    return (
        np.float32(contrastive),
        np.float32(match_loss),
        np.float32(nonmatch_loss),
    )
